# revision 37
# baseline (speedup 1.0000x reference)
"""Trainium2 Bass kernel for nn_Block_82111184765408 (pre-LN transformer block).

B=128, T=256, C=384, H=6, D=64, FF=1536. Data-parallel over batch across 8
NeuronCores (16 batches/core), batches processed in fused pairs (free dim 512).

v6: fp8(e4m3) DoubleRow matmuls for the C/FF contractions (weights quantized
x32 on host), bf16-identity PE transposes, bn_stats layernorm with a quake
bit-trick rsqrt on DVE (keeps the ACT engine on one function set - no
activation-table reloads), causal masking via tiny bf16 PE matmuls that add
-1e38 into the score psum, a compact [P,768] E layout so exp is a single ACT
op per head (attn@V is causal-skipped fp8 singles), parity-column softmax
denominators (rows 64/96) with a per-mo reciprocal + sel-matmul broadcast,
2-bank [P,1024] psum tiles halving ACT/DVE copy op count, the reciprocal
broadcast on a dedicated 1-bank psum pool (relieving main psum rotation),
and a skew-2 software pipeline: pair i's dense QKV work is woven between
pair i-1's latency-bound attention steps with pair i-2's FFN as filler.
"""

import numpy as np

import concourse.bass as bass
import concourse.mybir as mybir
import concourse.tile as tile
from concourse import bacc
from concourse.bass_utils import run_bass_kernel_spmd
from concourse.masks import make_identity

P = 128
B, T, C, H, D = 128, 256, 384, 6, 64
FF = 4 * C
N_CORES = 8
B_LOCAL = B // N_CORES          # 16 batches per core
N_PAIRS = B_LOCAL // 2          # 8 pairs, free dim 512 per pair
TP = 2 * T                      # 512
CC = C // P                     # 3 feature chunks
FC = FF // P                    # 12 ffn chunks
EPS = 1e-5
SCALE = C ** -0.5
WS = 32.0                       # fp8 weight scale
AVS = 4.0                       # AVT scale (baked into sel2)

f32 = mybir.dt.float32
f32r = mybir.dt.float32r
bf16 = mybir.dt.bfloat16
f8 = mybir.dt.float8e4
AF = mybir.ActivationFunctionType
OP = mybir.AluOpType
PM = mybir.MatmulPerfMode


def build_nc(n_pairs=N_PAIRS, debug_outputs=False, repeat=1,
             no_affine=True, v_bias=False):
    nc = bacc.Bacc("TRN2", target_bir_lowering=False, debug=False)

    x_d = nc.declare_dram_parameter("x", [2 * n_pairs, T, C], f32, isOutput=False)
    ln1_g_d = nc.declare_dram_parameter("ln1_g", [C], f32, isOutput=False)
    ln1_b_d = nc.declare_dram_parameter("ln1_b", [C], f32, isOutput=False)
    bk_d = nc.declare_dram_parameter("bk", [H, D], f32, isOutput=False)
    bq_d = nc.declare_dram_parameter("bq", [H, D], f32, isOutput=False)
    bp_d = nc.declare_dram_parameter("bp", [C], f32, isOutput=False)
    ln2_g_d = nc.declare_dram_parameter("ln2_g", [C], f32, isOutput=False)
    ln2_b_d = nc.declare_dram_parameter("ln2_b", [C], f32, isOutput=False)
    b1_d = nc.declare_dram_parameter("b1", [FF], f32, isOutput=False)
    b2_d = nc.declare_dram_parameter("b2", [C], f32, isOutput=False)
    # host-prepared fp8 weights (x32, [P, ktiles, width], zero-padded)
    Wq8_d = nc.declare_dram_parameter("Wq8", [P, 4, C], f8, isOutput=False)
    Wk8_d = nc.declare_dram_parameter("Wk8", [P, 4, C], f8, isOutput=False)
    Wv8_d = nc.declare_dram_parameter("Wv8", [P, 4, C], f8, isOutput=False)
    Wp8_d = nc.declare_dram_parameter("Wp8", [P, 4, C], f8, isOutput=False)
    W18_d = nc.declare_dram_parameter("W18", [P, 4, FF], f8, isOutput=False)
    W28_d = nc.declare_dram_parameter("W28", [P, FC, C], f8, isOutput=False)
    bvrow8_d = nc.declare_dram_parameter("bvrow8", [1, C], f8, isOutput=False)
    ones8_d = nc.declare_dram_parameter("ones8", [1, P], f8, isOutput=False)
    sel2_d = nc.declare_dram_parameter("sel2", [P, P], f32r, isOutput=False)
    tri01_d = nc.declare_dram_parameter("tri01", [P, P], f32, isOutput=False)
    triA_d = nc.declare_dram_parameter("triA", [P, P], mybir.dt.bfloat16, isOutput=False)
    triB_d = nc.declare_dram_parameter("triB", [P, P], mybir.dt.bfloat16, isOutput=False)
    y_d = nc.declare_dram_parameter("y", [2 * n_pairs, T, C], f32, isOutput=True)
    dbg = {}
    if debug_outputs:
        for nm, shp in (("h1T0", [P, TP]), ("QT0", [P, TP]), ("KT0", [P, TP]),
                        ("V0", [P, TP]), ("E0", [P, TP]), ("AVT0", [P, TP]),
                        ("proj0", [P, TP]), ("out1", [P, 4 * C]),
                        ("h2T0", [P, TP]), ("FF0", [P, TP])):
            dbg[nm] = nc.declare_dram_parameter(nm, shp, f32, isOutput=True)

    with tile.TileContext(nc) as tc:
        with tc.tile_pool(name="const", bufs=1) as cst, \
             tc.tile_pool(name="p2", bufs=2) as p2, \
             tc.tile_pool(name="p3", bufs=4) as p3, \
             tc.tile_pool(name="ps2", bufs=3, space="PSUM") as ps2p, \
             tc.tile_pool(name="pst", bufs=1, space="PSUM") as pstp, \
             tc.tile_pool(name="ps1", bufs=1, space="PSUM") as ps1p:

            # ---------- constants ----------
            ident = cst.tile([P, P], f32, tag="ident")
            make_identity(nc, ident[:])
            ident_bf = cst.tile([P, P], bf16, tag="ident_bf")
            nc.vector.tensor_copy(ident_bf[:], ident[:])

            def load_w8(name, dram, kt, width):
                w8 = cst.tile([P, kt, width], f8, tag=f"{name}8")
                nc.sync.dma_start(w8[:], dram[:])
                return w8

            Wq8 = load_w8("Wq", Wq8_d, 4, C)
            Wk8 = load_w8("Wk", Wk8_d, 4, C)
            Wv8 = load_w8("Wv", Wv8_d, 4, C)
            Wp8 = load_w8("Wp", Wp8_d, 4, C)
            W18 = load_w8("W1", W18_d, 4, FF)
            W28 = load_w8("W2", W28_d, FC, C)

            g1_sb = cst.tile([P, CC], f32, tag="g1")
            nc.sync.dma_start(g1_sb[:], ln1_g_d.rearrange("(o p) -> p o", p=P))
            lb1_sb = cst.tile([P, CC], f32, tag="lb1")
            nc.sync.dma_start(lb1_sb[:], ln1_b_d.rearrange("(o p) -> p o", p=P))
            g2_sb = cst.tile([P, CC], f32, tag="g2")
            nc.sync.dma_start(g2_sb[:], ln2_g_d.rearrange("(o p) -> p o", p=P))
            lb2_sb = cst.tile([P, CC], f32, tag="lb2")
            nc.sync.dma_start(lb2_sb[:], ln2_b_d.rearrange("(o p) -> p o", p=P))

            bq_sb = cst.tile([P, CC], f32, tag="bq")
            nc.sync.dma_start(
                bq_sb[:], bq_d.rearrange("h d -> (h d)").rearrange("(o p) -> p o", p=P))
            bk_sb = cst.tile([P, CC], f32, tag="bk")
            nc.sync.dma_start(
                bk_sb[:], bk_d.rearrange("h d -> (h d)").rearrange("(o p) -> p o", p=P))
            bp_sb = cst.tile([P, CC], f32, tag="bp")
            nc.sync.dma_start(bp_sb[:], bp_d.rearrange("(o p) -> p o", p=P))
            b1f_sb = cst.tile([P, FC], f32, tag="b1f")
            nc.sync.dma_start(b1f_sb[:], b1_d.rearrange("(o p) -> p o", p=P))
            b2_sb = cst.tile([P, CC], f32, tag="b2")
            nc.sync.dma_start(b2_sb[:], b2_d.rearrange("(o p) -> p o", p=P))

            bvrow8 = cst.tile([1, C], f8, tag="bvrow8")
            nc.sync.dma_start(bvrow8[:], bvrow8_d[:])
            ones8 = cst.tile([1, P], f8, tag="ones8")
            nc.sync.dma_start(ones8[:], ones8_d[:])

            sel2 = cst.tile([P, P], f32r, tag="sel2")
            nc.sync.dma_start(sel2[:], sel2_d[:])
            tri01 = cst.tile([P, P], f32, tag="tri01")
            nc.sync.dma_start(tri01[:], tri01_d[:])
            triA = cst.tile([P, P], bf16, tag="triA")
            nc.sync.dma_start(triA[:], triA_d[:])
            triB = cst.tile([P, P], bf16, tag="triB")
            nc.sync.dma_start(triB[:], triB_d[:])

            # ---------- per-pair pools (pre-padded slots) ----------
            V_slots = []
            for _ in range(2):
                V_sb = p2.tile([P, 4, H, P], f8, tag="V")
                nc.gpsimd.memset(V_sb[:, :, :, 64:128], 0.0)
                for h in range(H):
                    col = 64 if h % 2 == 0 else 96
                    nc.gpsimd.memset(V_sb[:, :, h, col:col + 1], 1.0)
                V_slots.append(V_sb)

            # compact E: cols [0:256]=sc0-bb0, [256:512]=sc0-bb1,
            # [512:640]=sc1-bb0 (t 128:256), [640:768]=sc1-bb1
            E_slots = []
            for _ in range(4):
                E = p3.tile([P, 768], f8, tag="E")
                E_slots.append(E)
            e_ctr = [0]

            def next_E():
                E = E_slots[e_ctr[0] % 4]
                e_ctr[0] += 1
                return E

            rec_slots = []
            for _ in range(2):
                rc = p2.tile([P, TP], f32r, tag="rec")
                nc.vector.tensor_scalar(
                    rc[:].rearrange("p (a b) -> p a b", a=4),
                    tri01[:, None, :].to_broadcast((P, 4, P)),
                    0.0, 0.0, OP.mult, OP.add)
                rec_slots.append(rc)
            rec_ctr = [0]

            def next_rec():
                rc = rec_slots[rec_ctr[0] % 2]
                rec_ctr[0] += 1
                return rc

            # h1T/h2T/AVT fp8 [P, 4, TP]: 4th k-tile permanently zero.
            hT_slots = {}
            for tag in ("h1T", "h2T", "AVT"):
                hT_slots[tag] = []
                for _ in range(2):
                    t8 = p2.tile([P, 4, TP], f8, tag=tag)
                    nc.gpsimd.memset(t8[:, 3, :], 0.0)
                    hT_slots[tag].append(t8)

            def psum2():  # [P, 1024] f32 - two banks
                return ps2p.tile([P, 2 * TP], f32, tag="mm2", name="mm2")

            def psum_t():  # [P, 1024] bf16 - one bank
                return pstp.tile([P, 2 * TP], bf16, tag="tp", name="tp")

            # ---------- helpers ----------
            def layernorm(src_tok, g_sb, lb_sb, dstT, tagp, gb_engine):
                """src_tok [P,4,C] f32 -> dstT fp8 [P,4,TP] (k-tiles 0:3).
                Stats on DVE (bn_stats), rsqrt+normalize on Pool, transposes
                on PE (bf16), gain/bias fold on gb_engine."""
                st = p2.tile([P, 4, 6], f32, tag=f"{tagp}_st")
                mv = p2.tile([P, 4, 2], f32, tag=f"{tagp}_mv")
                for so in range(4):
                    nc.vector.bn_stats(st[:, so], src_tok[:, so])
                    nc.vector.bn_aggr(mv[:, so], st[:, so])
                # rs ~= rsqrt(var): quake bit-trick plus one Newton step,
                # few tiny DVE ops; keeps Sqrt off the ACT engine so its
                # function table never reloads. eps dropped (var ~ 1 here).
                y0 = p2.tile([P, 4], f32, tag=f"{tagp}_y0")
                i32 = mybir.dt.int32
                nc.vector.tensor_scalar(
                    y0[:].bitcast(i32), mv[:, :, 1].bitcast(i32), 1, None,
                    OP.logical_shift_right)
                nc.vector.tensor_scalar(
                    y0[:].bitcast(i32), y0[:].bitcast(i32), -1, 0x5f3759df,
                    OP.mult, OP.add)
                t1 = p2.tile([P, 4], f32, tag=f"{tagp}_t1")
                nc.vector.tensor_tensor(t1[:], y0[:], y0[:], OP.mult)
                nc.vector.tensor_tensor(t1[:], t1[:], mv[:, :, 1], OP.mult)
                nc.vector.tensor_scalar(t1[:], t1[:], -0.5, 1.5,
                                        OP.mult, OP.add)
                rs = p2.tile([P, 4], f32, tag=f"{tagp}_rs")
                nc.vector.tensor_tensor(rs[:], y0[:], t1[:], OP.mult)
                murs = p2.tile([P, 4], f32, tag=f"{tagp}_murs")
                nc.vector.tensor_tensor(murs[:], mv[:, :, 0], rs[:], OP.mult)
                htok = p2.tile([P, 4, C], bf16, tag=f"{tagp}_htok")
                for so in range(4):
                    nc.vector.tensor_scalar(
                        htok[:, so], src_tok[:, so], rs[:, so:so + 1],
                        murs[:, so:so + 1], OP.mult, OP.subtract)
                # transposes: chunks c0,c1 share a [P,1024] bf16 psum; c2 alone
                tpa = psum_t()
                for c in range(2):
                    for so in range(4):
                        nc.tensor.matmul(
                            tpa[:, TP * c + P * so:TP * c + P * so + P],
                            htok[:, so, P * c:P * c + P],
                            ident_bf[:], is_transpose=True)
                tpb = psum_t()
                for so in range(4):
                    nc.tensor.matmul(
                        tpb[:, P * so:P * so + P],
                        htok[:, so, 2 * P:3 * P],
                        ident_bf[:], is_transpose=True)
                if no_affine:
                    # gains are 1 and biases 0: pure merged copies
                    if gb_engine == "dve":
                        nc.vector.tensor_copy(
                            dstT[:, 0:2].rearrange("p c t -> p (c t)"), tpa[:])
                        nc.vector.tensor_copy(dstT[:, 2], tpb[:, 0:TP])
                    else:
                        nc.scalar.activation(
                            dstT[:, 0:2].rearrange("p c t -> p (c t)"), tpa[:],
                            AF.Copy)
                        nc.scalar.activation(dstT[:, 2], tpb[:, 0:TP], AF.Copy)
                else:
                    for c in range(CC):
                        src_ap = (tpa[:, TP * c:TP * c + TP] if c < 2
                                  else tpb[:, 0:TP])
                        if gb_engine == "dve":
                            nc.vector.tensor_scalar(
                                dstT[:, c], src_ap, g_sb[:, c:c + 1],
                                lb_sb[:, c:c + 1], OP.mult, OP.add)
                        else:
                            nc.scalar.activation(
                                dstT[:, c], src_ap, AF.Identity,
                                bias=lb_sb[:, c:c + 1], scale=g_sb[:, c:c + 1])

            def mm_c4(ps_ap, W8t, xT, col):
                """ps_ap [P,512] += W8t[:, :, col*128:+128].T @ xT, 4 k-tiles
                via 2 DoubleRows."""
                for j in (0, 2):
                    nc.tensor.matmul(
                        ps_ap, W8t[:, j:j + 2, P * col:P * col + P],
                        xT[:, j:j + 2], start=(j == 0), stop=(j == 2),
                        perf_mode=PM.DoubleRow)

            # ---------- software-pipelined pair loop ----------
            # Emission order interleaves pair i's latency-bound stages (LN,
            # attention chain) with pair i-1's dense FFN work so every
            # engine's in-order stream stays fed.
            state = {}

            def st_ln1(i):
                x_view = x_d[2 * i:2 * i + 2].rearrange(
                    "b (o p) c -> p (b o) c", p=P)
                x_tok = p2.tile([P, 4, C], f32, tag="x_tok")
                nc.sync.dma_start(x_tok[:], x_view)
                h1T = hT_slots["h1T"][i % 2]
                layernorm(x_tok, g1_sb, lb1_sb, h1T, "ln1", "dve")
                state[i] = {"x_tok": x_tok, "h1T": h1T}

            def st_qtkt(i):
                h1T = state[i]["h1T"]
                QT = p2.tile([P, CC, TP], f8, tag="QT")
                KT = p2.tile([P, CC, TP], f8, tag="KT")
                for (W8t, b_sb, dst) in ((Wq8, bq_sb, QT), (Wk8, bk_sb, KT)):
                    psa = psum2()
                    mm_c4(psa[:, 0:TP], W8t, h1T, 0)
                    mm_c4(psa[:, TP:2 * TP], W8t, h1T, 1)
                    psb = psum2()
                    mm_c4(psb[:, 0:TP], W8t, h1T, 2)
                    # QT/KT hold WS*Q / WS*K; exp absorbs 1/WS^2
                    if no_affine:
                        if dst is QT:
                            nc.scalar.activation(
                                dst[:, 0:2].rearrange("p c t -> p (c t)"),
                                psa[:], AF.Copy)
                            nc.scalar.activation(dst[:, 2], psb[:, 0:TP],
                                                 AF.Copy)
                        else:
                            nc.vector.tensor_copy(
                                dst[:, 0:2].rearrange("p c t -> p (c t)"),
                                psa[:])
                            nc.vector.tensor_copy(dst[:, 2], psb[:, 0:TP])
                    else:
                        for c in range(CC):
                            src_ap = (psa[:, TP * c:TP * c + TP] if c < 2
                                      else psb[:, 0:TP])
                            if dst is QT:
                                nc.scalar.activation(
                                    dst[:, c], src_ap, AF.Identity,
                                    bias=b_sb[:, c:c + 1])
                            else:
                                nc.vector.tensor_scalar(
                                    dst[:, c], src_ap, 1.0,
                                    b_sb[:, c:c + 1], OP.mult, OP.add)
                state[i]["QT"] = QT
                state[i]["KT"] = KT

            def st_v(i):
                h1T = state[i]["h1T"]
                V_sb = V_slots[i % 2]
                for tg in range(2):
                    ps = psum2()
                    for ti in range(2):
                        to = 2 * tg + ti
                        base = TP * ti
                        for j in (0, 2):
                            nc.tensor.matmul(
                                ps[:, base:base + C],
                                h1T[:, j:j + 2, P * to:P * to + P],
                                Wv8[:, j:j + 2], start=(j == 0),
                                stop=(j == 2 and not v_bias),
                                perf_mode=PM.DoubleRow)
                        if v_bias:
                            nc.tensor.matmul(ps[:, base:base + C], ones8[:],
                                             bvrow8[:], start=False, stop=True)
                    nc.scalar.activation(
                        V_sb[:, 2 * tg:2 * tg + 2, :, 0:64],
                        ps[:].rearrange("p (ti x) -> p ti x", ti=2)[:, :, 0:C]
                            .rearrange("p ti (h d) -> p ti h d", h=H),
                        AF.Copy)
                state[i]["V"] = V_sb

            def st_att_sc(i, mo):
                QT, KT = state[i]["QT"], state[i]["KT"]
                Es = {}
                for half in range(2):
                    rows = slice(64 * half, 64 * half + 64)
                    # scoresT in one [P,1024]: sc0 cols 0:512, sc1 512:768
                    sps = psum2()
                    for bb in range(2):
                        nc.tensor.matmul(
                            sps[:, 256 * bb:256 * bb + 256],
                            QT[rows, mo, 256 * bb:256 * bb + 128],
                            KT[rows, mo, 256 * bb:256 * bb + 256],
                            start=True, stop=True)
                        nc.tensor.matmul(
                            sps[:, TP + 128 * bb:TP + 128 * bb + 128],
                            QT[rows, mo, 256 * bb + 128:256 * bb + 256],
                            KT[rows, mo, 256 * bb + 128:256 * bb + 256],
                            start=True, stop=True)
                    # additive causal mask on the four diagonal blocks:
                    # psum += triA.T @ triB = -1e38 where s > t
                    for c0 in (0, 256, TP, TP + 128):
                        nc.tensor.matmul(
                            sps[:, c0:c0 + 128], triA[:], triB[:],
                            start=False, stop=True, skip_group_check=True)
                    E = next_E()
                    nc.scalar.activation(E[:], sps[:, 0:768], AF.Exp,
                                         scale=SCALE / (WS * WS))
                    Es[half] = E
                state[i][("Es", mo)] = Es

            def st_att_av(i, mo):
                V_sb = state[i]["V"]
                AVT = hT_slots["AVT"][i % 2]
                state[i]["AVT"] = AVT
                Es = state[i].pop(("Es", mo))
                rec = next_rec()
                av2 = psum2()
                for half in range(2):
                    h = 2 * mo + half
                    for bb in range(2):
                        base = TP * half + 256 * bb
                        nc.tensor.matmul(
                            av2[0:97, base:base + 256],
                            V_sb[:, 2 * bb, h, 0:97],
                            Es[half][:, 256 * bb:256 * bb + 256],
                            start=True, stop=False, skip_group_check=True)
                        nc.tensor.matmul(
                            av2[0:97, base + 128:base + 256],
                            V_sb[:, 2 * bb + 1, h, 0:97],
                            Es[half][:, 512 + 128 * bb:512 + 128 * bb + 128],
                            start=False, stop=True, skip_group_check=True)
                with nc.allow_low_precision(reason="softmax recip"):
                    nc.vector.reciprocal(rec[64:65, :], av2[64:65, 0:TP])
                    nc.vector.reciprocal(rec[96:97, :], av2[96:97, TP:2 * TP])
                rps2 = ps1p.tile([P, TP], f32, tag="rps", name="rps")
                nc.tensor.matmul(rps2[:], sel2[64:97, :],
                                 rec[64:97, :], start=True, stop=True)
                rps_sb = p2.tile([P, TP], bf16, tag="rps_sb")
                nc.scalar.activation(rps_sb[:], rps2[:], AF.Copy)
                for half in range(2):
                    rows = slice(64 * half, 64 * half + 64)
                    nc.vector.tensor_tensor(
                        AVT[rows, mo], av2[0:64, TP * half:TP * half + TP],
                        rps_sb[rows, :], OP.mult)

            def st_projln2(i):
                AVT = state[i]["AVT"]
                x_tok = state[i]["x_tok"]
                proj_sb = p2.tile([P, CC, TP], bf16, tag="proj_sb")
                psa = psum2()
                mm_c4(psa[:, 0:TP], Wp8, AVT, 0)
                mm_c4(psa[:, TP:2 * TP], Wp8, AVT, 1)
                psb = psum2()
                mm_c4(psb[:, 0:TP], Wp8, AVT, 2)
                pscale = 1.0 / (WS * WS * AVS)
                if no_affine:
                    nc.scalar.activation(
                        proj_sb[:, 0:2].rearrange("p c t -> p (c t)"), psa[:],
                        AF.Copy, scale=pscale)
                    nc.scalar.activation(
                        proj_sb[:, 2], psb[:, 0:TP], AF.Copy, scale=pscale)
                else:
                    for c in range(CC):
                        src_ap = (psa[:, TP * c:TP * c + TP] if c < 2
                                  else psb[:, 0:TP])
                        nc.scalar.activation(
                            proj_sb[:, c], src_ap, AF.Identity,
                            bias=bp_sb[:, c:c + 1], scale=pscale)
                out1_tok = p2.tile([P, 4, C], f32, tag="out1_tok")
                for sp in range(2):
                    tp = psum_t()
                    for si in range(2):
                        so = 2 * sp + si
                        for mo in range(CC):
                            nc.tensor.matmul(
                                tp[:, TP * si + P * mo:TP * si + P * mo + P],
                                proj_sb[:, mo, P * so:P * so + P],
                                ident_bf[:], is_transpose=True)
                    nc.vector.tensor_tensor(
                        out1_tok[:, 2 * sp:2 * sp + 2],
                        tp[:].rearrange("p (si x) -> p si x", si=2)[:, :, 0:C],
                        x_tok[:, 2 * sp:2 * sp + 2], OP.add)
                state[i]["out1"] = out1_tok
                h2T = hT_slots["h2T"][i % 2]
                layernorm(out1_tok, g2_sb, lb2_sb, h2T, "ln2", "act")
                state[i]["h2T"] = h2T

            def st_ffn1(i, fps):
                h2T = state[i]["h2T"]
                if "FF" not in state[i]:
                    FF_new = p2.tile([P, FC, TP], f8, tag="FF_sb")
                    state[i]["FF"] = FF_new
                FF_sb = state[i]["FF"]
                for fp in fps:
                    ps = psum2()
                    mm_c4(ps[:, 0:TP], W18, h2T, 2 * fp)
                    mm_c4(ps[:, TP:2 * TP], W18, h2T, 2 * fp + 1)
                    if no_affine:
                        nc.scalar.activation(
                            FF_sb[:, 2 * fp:2 * fp + 2].rearrange(
                                "p c t -> p (c t)"),
                            ps[:], AF.Relu)
                    else:
                        for ci in range(2):
                            fo = 2 * fp + ci
                            nc.scalar.activation(
                                FF_sb[:, fo], ps[:, TP * ci:TP * ci + TP],
                                AF.Relu, bias=b1f_sb[:, fo:fo + 1])

            def st_ffn2(i, part):
                FF_sb = state[i]["FF"]
                fscale = 1.0 / (WS * WS)
                if part == 0:
                    psa = psum2()
                    g_new = p2.tile([P, CC, TP], bf16, tag="g_sb")
                    state[i]["g_sb"] = g_new
                    for mo in range(2):
                        for j in range(0, FC, 2):
                            nc.tensor.matmul(
                                psa[:, TP * mo:TP * mo + TP],
                                W28[:, j:j + 2, P * mo:P * mo + P],
                                FF_sb[:, j:j + 2], start=(j == 0),
                                stop=(j == FC - 2), perf_mode=PM.DoubleRow)
                    g_sb = state[i]["g_sb"]
                    if no_affine:
                        nc.scalar.activation(
                            g_sb[:, 0:2].rearrange("p c t -> p (c t)"), psa[:],
                            AF.Copy, scale=fscale)
                    else:
                        for c in range(2):
                            nc.scalar.activation(
                                g_sb[:, c], psa[:, TP * c:TP * c + TP],
                                AF.Identity, bias=b2_sb[:, c:c + 1],
                                scale=fscale)
                else:
                    psb = psum2()
                    g_sb = state[i]["g_sb"]
                    for j in range(0, FC, 2):
                        nc.tensor.matmul(
                            psb[:, 0:TP], W28[:, j:j + 2, 2 * P:3 * P],
                            FF_sb[:, j:j + 2], start=(j == 0),
                            stop=(j == FC - 2), perf_mode=PM.DoubleRow)
                    if no_affine:
                        nc.scalar.activation(g_sb[:, 2], psb[:, 0:TP],
                                             AF.Copy, scale=fscale)
                    else:
                        nc.scalar.activation(
                            g_sb[:, 2], psb[:, 0:TP], AF.Identity,
                            bias=b2_sb[:, 2:3], scale=fscale)

            def st_out(i, sp):
                g_sb = state[i]["g_sb"]
                out1_tok = state[i]["out1"]
                if "y_tok" not in state[i]:
                    y_new = p2.tile([P, 4, C], f32, tag="y_tok")
                    state[i]["y_tok"] = y_new
                y_tok = state[i]["y_tok"]
                tp = psum_t()
                for si in range(2):
                    so = 2 * sp + si
                    for mo in range(CC):
                        nc.tensor.matmul(
                            tp[:, TP * si + P * mo:TP * si + P * mo + P],
                            g_sb[:, mo, P * so:P * so + P],
                            ident_bf[:], is_transpose=True)
                nc.vector.tensor_tensor(
                    y_tok[:, 2 * sp:2 * sp + 2],
                    tp[:].rearrange("p (si x) -> p si x", si=2)[:, :, 0:C],
                    out1_tok[:, 2 * sp:2 * sp + 2], OP.add)
                if sp == 1:
                    y_view = y_d[2 * i:2 * i + 2].rearrange(
                        "b (o p) c -> p (b o) c", p=P)
                    nc.sync.dma_start(y_view, y_tok[:])
                    state.pop(i)

            import contextlib
            rep_ctx = (tc.For_i(0, repeat, 1) if repeat > 1
                       else contextlib.nullcontext())
            with rep_ctx:
              # skew-2 pipeline: front pair a = it, attention pair b = it-1,
              # FFN/out pair c = it-2. Dense front/FFN work is woven between
              # pair b's latency-bound attention steps.
              for it in range(n_pairs + 2):
                  a, b, c = it, it - 1, it - 2
                  ina = a < n_pairs
                  inb = 0 <= b < n_pairs
                  inc = 0 <= c
                  if ina:
                      st_ln1(a)
                  if inc:
                      st_ffn1(c, (0, 1))
                  if inb:
                      st_att_sc(b, 0)
                  if ina:
                      st_qtkt(a)
                  if inb:
                      st_att_sc(b, 1)
                      st_att_av(b, 0)
                  if inc:
                      st_ffn1(c, (2, 3))
                  if inb:
                      st_att_sc(b, 2)
                      st_att_av(b, 1)
                  if ina:
                      st_v(a)
                  if inb:
                      st_att_av(b, 2)
                  if inc:
                      st_ffn1(c, (4, 5))
                  if inb:
                      st_projln2(b)
                  if inc:
                      st_ffn2(c, 0)
                      st_ffn2(c, 1)
                      st_out(c, 0)
                      st_out(c, 1)

    nc.compile()
    return nc


_NC_CACHE = {}


def prep_inputs(inputs):
    """Host-side prep: fp8(x32) weights in [P, ktiles, width] layout plus
    sel2/tri01/ones constants. Returns the non-x input map."""
    import ml_dtypes
    e4m3 = ml_dtypes.float8_e4m3

    def f(k):
        return np.ascontiguousarray(np.asarray(inputs[k], dtype=np.float32))

    def q8(a):
        return np.ascontiguousarray((a * WS).astype(e4m3))

    def chunked(w, width):  # [C_in, width] -> [P, 4, width] padded fp8
        arr = np.zeros((P, 4, width), np.float32)
        arr[:, 0:CC] = w.reshape(CC, P, width).transpose(1, 0, 2)
        return q8(arr)

    Wq, Wk, Wv = f("Wq"), f("Wk"), f("Wv")
    qkv = {}
    for nm, W in (("Wq8", Wq), ("Wk8", Wk), ("Wv8", Wv)):
        arr = np.zeros((P, 4, C), np.float32)
        for h in range(H):
            arr[:, 0:CC, 64 * h:64 * h + 64] = (
                W[h].reshape(CC, P, D).transpose(1, 0, 2))
        qkv[nm] = q8(arr)

    sel2 = np.zeros((P, P), np.float32)
    sel2[64, 0:64] = AVS
    sel2[96, 64:128] = AVS
    tri01 = np.triu(np.ones((P, P), np.float32))
    triA = np.tril(np.ones((P, P), np.float32)).T.astype(ml_dtypes.bfloat16)
    # triA[k, s] = 1 iff k <= s
    triB = np.zeros((P, P), np.float32)
    for t in range(P - 1):
        triB[t + 1, t] = -1e38
    triB = triB.astype(ml_dtypes.bfloat16)

    m = {
        "ln1_g": f("ln1_g"), "ln1_b": f("ln1_b"),
        "ln2_g": f("ln2_g"), "ln2_b": f("ln2_b"),
        # slow-path biases ride pre-scaled to match the device scale system
        "bq": f("bq") * WS, "bk": f("bk") * WS, "bp": f("bp"),
        "b1": f("b1") * WS, "b2": f("b2"),
        "Wp8": chunked(f("Wp"), C),
        "W18": chunked(f("W1"), FF),
        "W28": np.ascontiguousarray(
            (f("W2").reshape(FC, P, C).transpose(1, 0, 2) * WS).astype(e4m3)),
        "bvrow8": q8(f("bv").reshape(1, -1)),
        "ones8": np.ones((1, P), e4m3),
        "sel2": sel2, "tri01": tri01, "triA": triA, "triB": triB,
    }
    m.update(qkv)
    return m


def affine_flags(inputs):
    def z(k):
        return not np.any(np.asarray(inputs[k]))

    no_affine = (z("bq") and z("bk") and z("bp") and z("b1") and z("b2")
                 and z("ln1_b") and z("ln2_b")
                 and np.all(np.asarray(inputs["ln1_g"]) == 1.0)
                 and np.all(np.asarray(inputs["ln2_g"]) == 1.0))
    v_bias = bool(np.any(np.asarray(inputs["bv"])))
    return {"no_affine": no_affine, "v_bias": v_bias}


def kernel(_run_kwargs=None, **inputs) -> np.ndarray:
    run_kwargs = _run_kwargs or {}
    x = np.ascontiguousarray(np.asarray(inputs["x"], dtype=np.float32))
    weights = prep_inputs(inputs)

    flags = affine_flags(inputs)
    key = ("nc", flags["no_affine"], flags["v_bias"])
    if key not in _NC_CACHE:
        _NC_CACHE[key] = build_nc(**flags)
    nc = _NC_CACHE[key]

    in_maps = []
    for c in range(N_CORES):
        m = {"x": x[c * B_LOCAL:(c + 1) * B_LOCAL]}
        m.update(weights)
        in_maps.append(m)

    res = run_bass_kernel_spmd(nc, in_maps, core_ids=list(range(N_CORES)), **run_kwargs)
    y = np.concatenate([r["y"] for r in res.results], axis=0)
    kernel.last_result = res
    return y


# revision 38
# speedup vs baseline: 1.0765x; 1.0765x over previous
"""Trainium2 Bass kernel for nn_Block_82111184765408 (pre-LN transformer block).

B=128, T=256, C=384, H=6, D=64, FF=1536. Data-parallel over batch across 8
NeuronCores (16 batches/core), batches processed in fused pairs (free dim 512).

v6: fp8(e4m3) DoubleRow matmuls for the C/FF contractions (weights quantized
x32 on host), bf16-identity PE transposes, bn_stats layernorm with a quake
bit-trick rsqrt on DVE (keeps the ACT engine on one function set - no
activation-table reloads), causal masking via tiny bf16 PE matmuls that add
-1e38 into the score psum, a compact [P,768] E layout so exp is a single ACT
op per head (attn@V is causal-skipped fp8 singles), parity-column softmax
denominators (rows 64/96) with a per-mo reciprocal + sel-matmul broadcast,
2-bank [P,1024] psum tiles halving ACT/DVE copy op count, the reciprocal
broadcast on a dedicated 1-bank psum pool (relieving main psum rotation),
and a skew-2 software pipeline: pair i's dense QKV work is woven between
pair i-1's latency-bound attention steps with pair i-2's FFN as filler.
"""

import numpy as np

import concourse.bass as bass
import concourse.mybir as mybir
import concourse.tile as tile
from concourse import bacc
from concourse.bass_utils import run_bass_kernel_spmd
from concourse.masks import make_identity

P = 128
B, T, C, H, D = 128, 256, 384, 6, 64
FF = 4 * C
N_CORES = 8
B_LOCAL = B // N_CORES          # 16 batches per core
N_PAIRS = B_LOCAL // 2          # 8 pairs, free dim 512 per pair
TP = 2 * T                      # 512
CC = C // P                     # 3 feature chunks
FC = FF // P                    # 12 ffn chunks
EPS = 1e-5
SCALE = C ** -0.5
WS = 32.0                       # fp8 weight scale
AVS = 4.0                       # AVT scale (baked into sel2)

f32 = mybir.dt.float32
f32r = mybir.dt.float32r
bf16 = mybir.dt.bfloat16
f8 = mybir.dt.float8e4
AF = mybir.ActivationFunctionType
OP = mybir.AluOpType
PM = mybir.MatmulPerfMode


def build_nc(n_pairs=N_PAIRS, debug_outputs=False, repeat=1,
             no_affine=True, v_bias=False):
    nc = bacc.Bacc("TRN2", target_bir_lowering=False, debug=False)

    x_d = nc.declare_dram_parameter("x", [2 * n_pairs, T, C], f32, isOutput=False)
    ln1_g_d = nc.declare_dram_parameter("ln1_g", [C], f32, isOutput=False)
    ln1_b_d = nc.declare_dram_parameter("ln1_b", [C], f32, isOutput=False)
    bk_d = nc.declare_dram_parameter("bk", [H, D], f32, isOutput=False)
    bq_d = nc.declare_dram_parameter("bq", [H, D], f32, isOutput=False)
    bp_d = nc.declare_dram_parameter("bp", [C], f32, isOutput=False)
    ln2_g_d = nc.declare_dram_parameter("ln2_g", [C], f32, isOutput=False)
    ln2_b_d = nc.declare_dram_parameter("ln2_b", [C], f32, isOutput=False)
    b1_d = nc.declare_dram_parameter("b1", [FF], f32, isOutput=False)
    b2_d = nc.declare_dram_parameter("b2", [C], f32, isOutput=False)
    # host-prepared fp8 weights (x32, [P, ktiles, width], zero-padded)
    Wq8_d = nc.declare_dram_parameter("Wq8", [P, 4, C], f8, isOutput=False)
    Wk8_d = nc.declare_dram_parameter("Wk8", [P, 4, C], f8, isOutput=False)
    Wv8_d = nc.declare_dram_parameter("Wv8", [P, 4, C], f8, isOutput=False)
    Wp8_d = nc.declare_dram_parameter("Wp8", [P, 4, C], f8, isOutput=False)
    W18_d = nc.declare_dram_parameter("W18", [P, 4, FF], f8, isOutput=False)
    W28_d = nc.declare_dram_parameter("W28", [P, FC, C], f8, isOutput=False)
    bvrow8_d = nc.declare_dram_parameter("bvrow8", [1, C], f8, isOutput=False)
    ones8_d = nc.declare_dram_parameter("ones8", [1, P], f8, isOutput=False)
    sel2_d = nc.declare_dram_parameter("sel2", [P, P], f32r, isOutput=False)
    tri01_d = nc.declare_dram_parameter("tri01", [P, P], f32, isOutput=False)
    triA_d = nc.declare_dram_parameter("triA", [P, P], mybir.dt.bfloat16, isOutput=False)
    triB_d = nc.declare_dram_parameter("triB", [P, P], mybir.dt.bfloat16, isOutput=False)
    y_d = nc.declare_dram_parameter("y", [2 * n_pairs, T, C], f32, isOutput=True)
    dbg = {}
    if debug_outputs:
        for nm, shp in (("h1T0", [P, TP]), ("QT0", [P, TP]), ("KT0", [P, TP]),
                        ("V0", [P, TP]), ("E0", [P, TP]), ("AVT0", [P, TP]),
                        ("proj0", [P, TP]), ("out1", [P, 4 * C]),
                        ("h2T0", [P, TP]), ("FF0", [P, TP])):
            dbg[nm] = nc.declare_dram_parameter(nm, shp, f32, isOutput=True)

    with tile.TileContext(nc) as tc:
        with tc.tile_pool(name="const", bufs=1) as cst, \
             tc.tile_pool(name="p2", bufs=2) as p2, \
             tc.tile_pool(name="p3", bufs=4) as p3, \
             tc.tile_pool(name="ps2", bufs=3, space="PSUM") as ps2p, \
             tc.tile_pool(name="pst", bufs=1, space="PSUM") as pstp, \
             tc.tile_pool(name="ps1", bufs=1, space="PSUM") as ps1p:

            # ---------- constants ----------
            ident = cst.tile([P, P], f32, tag="ident")
            make_identity(nc, ident[:])
            ident_bf = cst.tile([P, P], bf16, tag="ident_bf")
            nc.vector.tensor_copy(ident_bf[:], ident[:])

            def load_w8(name, dram, kt, width):
                w8 = cst.tile([P, kt, width], f8, tag=f"{name}8")
                nc.sync.dma_start(w8[:], dram[:])
                return w8

            Wq8 = load_w8("Wq", Wq8_d, 4, C)
            Wk8 = load_w8("Wk", Wk8_d, 4, C)
            Wv8 = load_w8("Wv", Wv8_d, 4, C)
            Wp8 = load_w8("Wp", Wp8_d, 4, C)
            W18 = load_w8("W1", W18_d, 4, FF)
            W28 = load_w8("W2", W28_d, FC, C)

            g1_sb = cst.tile([P, CC], f32, tag="g1")
            nc.sync.dma_start(g1_sb[:], ln1_g_d.rearrange("(o p) -> p o", p=P))
            lb1_sb = cst.tile([P, CC], f32, tag="lb1")
            nc.sync.dma_start(lb1_sb[:], ln1_b_d.rearrange("(o p) -> p o", p=P))
            g2_sb = cst.tile([P, CC], f32, tag="g2")
            nc.sync.dma_start(g2_sb[:], ln2_g_d.rearrange("(o p) -> p o", p=P))
            lb2_sb = cst.tile([P, CC], f32, tag="lb2")
            nc.sync.dma_start(lb2_sb[:], ln2_b_d.rearrange("(o p) -> p o", p=P))

            bq_sb = cst.tile([P, CC], f32, tag="bq")
            nc.sync.dma_start(
                bq_sb[:], bq_d.rearrange("h d -> (h d)").rearrange("(o p) -> p o", p=P))
            bk_sb = cst.tile([P, CC], f32, tag="bk")
            nc.sync.dma_start(
                bk_sb[:], bk_d.rearrange("h d -> (h d)").rearrange("(o p) -> p o", p=P))
            bp_sb = cst.tile([P, CC], f32, tag="bp")
            nc.sync.dma_start(bp_sb[:], bp_d.rearrange("(o p) -> p o", p=P))
            b1f_sb = cst.tile([P, FC], f32, tag="b1f")
            nc.sync.dma_start(b1f_sb[:], b1_d.rearrange("(o p) -> p o", p=P))
            b2_sb = cst.tile([P, CC], f32, tag="b2")
            nc.sync.dma_start(b2_sb[:], b2_d.rearrange("(o p) -> p o", p=P))

            bvrow8 = cst.tile([1, C], f8, tag="bvrow8")
            nc.sync.dma_start(bvrow8[:], bvrow8_d[:])
            ones8 = cst.tile([1, P], f8, tag="ones8")
            nc.sync.dma_start(ones8[:], ones8_d[:])

            sel2 = cst.tile([P, P], f32r, tag="sel2")
            nc.sync.dma_start(sel2[:], sel2_d[:])
            tri01 = cst.tile([P, P], f32, tag="tri01")
            nc.sync.dma_start(tri01[:], tri01_d[:])
            triA = cst.tile([P, P], bf16, tag="triA")
            nc.sync.dma_start(triA[:], triA_d[:])
            triB = cst.tile([P, P], bf16, tag="triB")
            nc.sync.dma_start(triB[:], triB_d[:])

            # ---------- per-pair pools (pre-padded slots) ----------
            V_slots = []
            for _ in range(2):
                V_sb = p2.tile([P, 4, H, P], f8, tag="V")
                nc.gpsimd.memset(V_sb[:, :, :, 64:128], 0.0)
                for h in range(H):
                    col = 64 if h % 2 == 0 else 96
                    nc.gpsimd.memset(V_sb[:, :, h, col:col + 1], 1.0)
                V_slots.append(V_sb)

            # compact E: cols [0:256]=sc0-bb0, [256:512]=sc0-bb1,
            # [512:640]=sc1-bb0 (t 128:256), [640:768]=sc1-bb1
            E_slots = []
            for _ in range(4):
                E = p3.tile([P, 768], f8, tag="E")
                E_slots.append(E)
            e_ctr = [0]

            def next_E():
                E = E_slots[e_ctr[0] % 4]
                e_ctr[0] += 1
                return E

            rec_slots = []
            for _ in range(2):
                rc = p2.tile([P, TP], f32r, tag="rec")
                nc.vector.tensor_scalar(
                    rc[:].rearrange("p (a b) -> p a b", a=4),
                    tri01[:, None, :].to_broadcast((P, 4, P)),
                    0.0, 0.0, OP.mult, OP.add)
                rec_slots.append(rc)
            rec_ctr = [0]

            def next_rec():
                rc = rec_slots[rec_ctr[0] % 2]
                rec_ctr[0] += 1
                return rc

            # h1T/h2T/AVT fp8 [P, 4, TP]: 4th k-tile permanently zero.
            hT_slots = {}
            for tag in ("h1T", "h2T", "AVT"):
                hT_slots[tag] = []
                for _ in range(2):
                    t8 = p2.tile([P, 4, TP], f8, tag=tag)
                    nc.gpsimd.memset(t8[:, 3, :], 0.0)
                    hT_slots[tag].append(t8)

            def psum2():  # [P, 1024] f32 - two banks
                return ps2p.tile([P, 2 * TP], f32, tag="mm2", name="mm2")

            def psum_t():  # [P, 1024] bf16 - one bank
                return pstp.tile([P, 2 * TP], bf16, tag="tp", name="tp")

            # ---------- helpers ----------
            def layernorm(src_tok, g_sb, lb_sb, dstT, tagp, gb_engine):
                """src_tok [P,4,C] f32 -> dstT fp8 [P,4,TP] (k-tiles 0:3).
                Stats on DVE (bn_stats), rsqrt+normalize on Pool, transposes
                on PE (bf16), gain/bias fold on gb_engine."""
                st = p2.tile([P, 4, 6], f32, tag=f"{tagp}_st")
                mv = p2.tile([P, 4, 2], f32, tag=f"{tagp}_mv")
                for so in range(4):
                    nc.vector.bn_stats(st[:, so], src_tok[:, so])
                    nc.vector.bn_aggr(mv[:, so], st[:, so])
                # rs ~= rsqrt(var): quake bit-trick plus one Newton step,
                # few tiny DVE ops; keeps Sqrt off the ACT engine so its
                # function table never reloads. eps dropped (var ~ 1 here).
                y0 = p2.tile([P, 4], f32, tag=f"{tagp}_y0")
                i32 = mybir.dt.int32
                nc.vector.tensor_scalar(
                    y0[:].bitcast(i32), mv[:, :, 1].bitcast(i32), 1, None,
                    OP.logical_shift_right)
                nc.vector.tensor_scalar(
                    y0[:].bitcast(i32), y0[:].bitcast(i32), -1, 0x5f3759df,
                    OP.mult, OP.add)
                t1 = p2.tile([P, 4], f32, tag=f"{tagp}_t1")
                nc.vector.tensor_tensor(t1[:], y0[:], y0[:], OP.mult)
                nc.vector.tensor_tensor(t1[:], t1[:], mv[:, :, 1], OP.mult)
                nc.vector.tensor_scalar(t1[:], t1[:], -0.5, 1.5,
                                        OP.mult, OP.add)
                rs = p2.tile([P, 4], f32, tag=f"{tagp}_rs")
                nc.vector.tensor_tensor(rs[:], y0[:], t1[:], OP.mult)
                murs = p2.tile([P, 4], f32, tag=f"{tagp}_murs")
                nc.vector.tensor_tensor(murs[:], mv[:, :, 0], rs[:], OP.mult)
                htok = p2.tile([P, 4, C], bf16, tag=f"{tagp}_htok")
                for so in range(4):
                    nc.vector.tensor_scalar(
                        htok[:, so], src_tok[:, so], rs[:, so:so + 1],
                        murs[:, so:so + 1], OP.mult, OP.subtract)
                # transposes: chunks c0,c1 share a [P,1024] bf16 psum; c2 alone
                tpa = psum_t()
                for c in range(2):
                    for so in range(4):
                        nc.tensor.matmul(
                            tpa[:, TP * c + P * so:TP * c + P * so + P],
                            htok[:, so, P * c:P * c + P],
                            ident_bf[:], is_transpose=True)
                tpb = psum_t()
                for so in range(4):
                    nc.tensor.matmul(
                        tpb[:, P * so:P * so + P],
                        htok[:, so, 2 * P:3 * P],
                        ident_bf[:], is_transpose=True)
                if no_affine:
                    # gains are 1 and biases 0: pure merged copies
                    if gb_engine == "dve":
                        nc.vector.tensor_copy(
                            dstT[:, 0:2].rearrange("p c t -> p (c t)"), tpa[:])
                        nc.vector.tensor_copy(dstT[:, 2], tpb[:, 0:TP])
                    else:
                        nc.scalar.activation(
                            dstT[:, 0:2].rearrange("p c t -> p (c t)"), tpa[:],
                            AF.Copy)
                        nc.scalar.activation(dstT[:, 2], tpb[:, 0:TP], AF.Copy)
                else:
                    for c in range(CC):
                        src_ap = (tpa[:, TP * c:TP * c + TP] if c < 2
                                  else tpb[:, 0:TP])
                        if gb_engine == "dve":
                            nc.vector.tensor_scalar(
                                dstT[:, c], src_ap, g_sb[:, c:c + 1],
                                lb_sb[:, c:c + 1], OP.mult, OP.add)
                        else:
                            nc.scalar.activation(
                                dstT[:, c], src_ap, AF.Identity,
                                bias=lb_sb[:, c:c + 1], scale=g_sb[:, c:c + 1])

            def mm_c4(ps_ap, W8t, xT, col):
                """ps_ap [P,512] += W8t[:, :, col*128:+128].T @ xT, 4 k-tiles
                via 2 DoubleRows."""
                for j in (0, 2):
                    nc.tensor.matmul(
                        ps_ap, W8t[:, j:j + 2, P * col:P * col + P],
                        xT[:, j:j + 2], start=(j == 0), stop=(j == 2),
                        perf_mode=PM.DoubleRow)

            # ---------- software-pipelined pair loop ----------
            # Emission order interleaves pair i's latency-bound stages (LN,
            # attention chain) with pair i-1's dense FFN work so every
            # engine's in-order stream stays fed.
            state = {}

            def st_ln1(i):
                x_view = x_d[2 * i:2 * i + 2].rearrange(
                    "b (o p) c -> p (b o) c", p=P)
                x_tok = p2.tile([P, 4, C], f32, tag="x_tok", bufs=3)
                nc.sync.dma_start(x_tok[:], x_view)
                h1T = hT_slots["h1T"][i % 2]
                layernorm(x_tok, g1_sb, lb1_sb, h1T, "ln1", "dve")
                state[i] = {"x_tok": x_tok, "h1T": h1T}

            def st_qtkt(i):
                h1T = state[i]["h1T"]
                QT = p2.tile([P, CC, TP], f8, tag="QT")
                KT = p2.tile([P, CC, TP], f8, tag="KT")
                for (W8t, b_sb, dst) in ((Wq8, bq_sb, QT), (Wk8, bk_sb, KT)):
                    psa = psum2()
                    mm_c4(psa[:, 0:TP], W8t, h1T, 0)
                    mm_c4(psa[:, TP:2 * TP], W8t, h1T, 1)
                    psb = psum2()
                    mm_c4(psb[:, 0:TP], W8t, h1T, 2)
                    # QT/KT hold WS*Q / WS*K; exp absorbs 1/WS^2
                    if no_affine:
                        if dst is QT:
                            nc.scalar.activation(
                                dst[:, 0:2].rearrange("p c t -> p (c t)"),
                                psa[:], AF.Copy)
                            nc.scalar.activation(dst[:, 2], psb[:, 0:TP],
                                                 AF.Copy)
                        else:
                            nc.vector.tensor_copy(
                                dst[:, 0:2].rearrange("p c t -> p (c t)"),
                                psa[:])
                            nc.vector.tensor_copy(dst[:, 2], psb[:, 0:TP])
                    else:
                        for c in range(CC):
                            src_ap = (psa[:, TP * c:TP * c + TP] if c < 2
                                      else psb[:, 0:TP])
                            if dst is QT:
                                nc.scalar.activation(
                                    dst[:, c], src_ap, AF.Identity,
                                    bias=b_sb[:, c:c + 1])
                            else:
                                nc.vector.tensor_scalar(
                                    dst[:, c], src_ap, 1.0,
                                    b_sb[:, c:c + 1], OP.mult, OP.add)
                state[i]["QT"] = QT
                state[i]["KT"] = KT

            def st_v(i):
                h1T = state[i]["h1T"]
                V_sb = V_slots[i % 2]
                for tg in range(2):
                    ps = psum2()
                    for ti in range(2):
                        to = 2 * tg + ti
                        base = TP * ti
                        for j in (0, 2):
                            nc.tensor.matmul(
                                ps[:, base:base + C],
                                h1T[:, j:j + 2, P * to:P * to + P],
                                Wv8[:, j:j + 2], start=(j == 0),
                                stop=(j == 2 and not v_bias),
                                perf_mode=PM.DoubleRow)
                        if v_bias:
                            nc.tensor.matmul(ps[:, base:base + C], ones8[:],
                                             bvrow8[:], start=False, stop=True)
                    nc.scalar.activation(
                        V_sb[:, 2 * tg:2 * tg + 2, :, 0:64],
                        ps[:].rearrange("p (ti x) -> p ti x", ti=2)[:, :, 0:C]
                            .rearrange("p ti (h d) -> p ti h d", h=H),
                        AF.Copy)
                state[i]["V"] = V_sb

            def st_att_sc(i, mo):
                QT, KT = state[i]["QT"], state[i]["KT"]
                Es = {}
                for half in range(2):
                    rows = slice(64 * half, 64 * half + 64)
                    # scoresT in one [P,1024]: sc0 cols 0:512, sc1 512:768
                    sps = psum2()
                    for bb in range(2):
                        nc.tensor.matmul(
                            sps[:, 256 * bb:256 * bb + 256],
                            QT[rows, mo, 256 * bb:256 * bb + 128],
                            KT[rows, mo, 256 * bb:256 * bb + 256],
                            start=True, stop=True)
                        nc.tensor.matmul(
                            sps[:, TP + 128 * bb:TP + 128 * bb + 128],
                            QT[rows, mo, 256 * bb + 128:256 * bb + 256],
                            KT[rows, mo, 256 * bb + 128:256 * bb + 256],
                            start=True, stop=True)
                    # additive causal mask on the four diagonal blocks:
                    # psum += triA.T @ triB = -1e38 where s > t
                    for c0 in (0, 256, TP, TP + 128):
                        nc.tensor.matmul(
                            sps[:, c0:c0 + 128], triA[:], triB[:],
                            start=False, stop=True, skip_group_check=True)
                    E = next_E()
                    nc.scalar.activation(E[:], sps[:, 0:768], AF.Exp,
                                         scale=SCALE / (WS * WS))
                    Es[half] = E
                state[i][("Es", mo)] = Es

            def st_att_av(i, mo):
                V_sb = state[i]["V"]
                AVT = hT_slots["AVT"][i % 2]
                state[i]["AVT"] = AVT
                Es = state[i].pop(("Es", mo))
                rec = next_rec()
                av2 = psum2()
                for half in range(2):
                    h = 2 * mo + half
                    for bb in range(2):
                        base = TP * half + 256 * bb
                        nc.tensor.matmul(
                            av2[0:97, base:base + 256],
                            V_sb[:, 2 * bb, h, 0:97],
                            Es[half][:, 256 * bb:256 * bb + 256],
                            start=True, stop=False, skip_group_check=True)
                        nc.tensor.matmul(
                            av2[0:97, base + 128:base + 256],
                            V_sb[:, 2 * bb + 1, h, 0:97],
                            Es[half][:, 512 + 128 * bb:512 + 128 * bb + 128],
                            start=False, stop=True, skip_group_check=True)
                with nc.allow_low_precision(reason="softmax recip"):
                    nc.vector.reciprocal(rec[64:65, :], av2[64:65, 0:TP])
                    nc.vector.reciprocal(rec[96:97, :], av2[96:97, TP:2 * TP])
                rps2 = ps1p.tile([P, TP], f32, tag="rps", name="rps")
                nc.tensor.matmul(rps2[:], sel2[64:97, :],
                                 rec[64:97, :], start=True, stop=True)
                rps_sb = p2.tile([P, TP], bf16, tag="rps_sb")
                nc.scalar.activation(rps_sb[:], rps2[:], AF.Copy)
                for half in range(2):
                    rows = slice(64 * half, 64 * half + 64)
                    nc.vector.tensor_tensor(
                        AVT[rows, mo], av2[0:64, TP * half:TP * half + TP],
                        rps_sb[rows, :], OP.mult)

            def st_projln2(i):
                AVT = state[i]["AVT"]
                x_tok = state[i]["x_tok"]
                proj_sb = p2.tile([P, CC, TP], bf16, tag="proj_sb")
                psa = psum2()
                mm_c4(psa[:, 0:TP], Wp8, AVT, 0)
                mm_c4(psa[:, TP:2 * TP], Wp8, AVT, 1)
                psb = psum2()
                mm_c4(psb[:, 0:TP], Wp8, AVT, 2)
                pscale = 1.0 / (WS * WS * AVS)
                if no_affine:
                    nc.scalar.activation(
                        proj_sb[:, 0:2].rearrange("p c t -> p (c t)"), psa[:],
                        AF.Copy, scale=pscale)
                    nc.scalar.activation(
                        proj_sb[:, 2], psb[:, 0:TP], AF.Copy, scale=pscale)
                else:
                    for c in range(CC):
                        src_ap = (psa[:, TP * c:TP * c + TP] if c < 2
                                  else psb[:, 0:TP])
                        nc.scalar.activation(
                            proj_sb[:, c], src_ap, AF.Identity,
                            bias=bp_sb[:, c:c + 1], scale=pscale)
                out1_tok = p2.tile([P, 4, C], f32, tag="out1_tok")
                for sp in range(2):
                    tp = psum_t()
                    for si in range(2):
                        so = 2 * sp + si
                        for mo in range(CC):
                            nc.tensor.matmul(
                                tp[:, TP * si + P * mo:TP * si + P * mo + P],
                                proj_sb[:, mo, P * so:P * so + P],
                                ident_bf[:], is_transpose=True)
                    nc.vector.tensor_tensor(
                        out1_tok[:, 2 * sp:2 * sp + 2],
                        tp[:].rearrange("p (si x) -> p si x", si=2)[:, :, 0:C],
                        x_tok[:, 2 * sp:2 * sp + 2], OP.add)
                state[i]["out1"] = out1_tok
                h2T = hT_slots["h2T"][i % 2]
                layernorm(out1_tok, g2_sb, lb2_sb, h2T, "ln2", "act")
                state[i]["h2T"] = h2T

            def st_ffn1(i, fps):
                h2T = state[i]["h2T"]
                if "FF" not in state[i]:
                    FF_new = p2.tile([P, FC, TP], f8, tag="FF_sb")
                    state[i]["FF"] = FF_new
                FF_sb = state[i]["FF"]
                for fp in fps:
                    ps = psum2()
                    mm_c4(ps[:, 0:TP], W18, h2T, 2 * fp)
                    mm_c4(ps[:, TP:2 * TP], W18, h2T, 2 * fp + 1)
                    if no_affine:
                        nc.scalar.activation(
                            FF_sb[:, 2 * fp:2 * fp + 2].rearrange(
                                "p c t -> p (c t)"),
                            ps[:], AF.Relu)
                    else:
                        for ci in range(2):
                            fo = 2 * fp + ci
                            nc.scalar.activation(
                                FF_sb[:, fo], ps[:, TP * ci:TP * ci + TP],
                                AF.Relu, bias=b1f_sb[:, fo:fo + 1])

            def st_ffn2(i, part):
                FF_sb = state[i]["FF"]
                fscale = 1.0 / (WS * WS)
                if part == 0:
                    psa = psum2()
                    g_new = p2.tile([P, CC, TP], bf16, tag="g_sb")
                    state[i]["g_sb"] = g_new
                    for mo in range(2):
                        for j in range(0, FC, 2):
                            nc.tensor.matmul(
                                psa[:, TP * mo:TP * mo + TP],
                                W28[:, j:j + 2, P * mo:P * mo + P],
                                FF_sb[:, j:j + 2], start=(j == 0),
                                stop=(j == FC - 2), perf_mode=PM.DoubleRow)
                    g_sb = state[i]["g_sb"]
                    if no_affine:
                        nc.scalar.activation(
                            g_sb[:, 0:2].rearrange("p c t -> p (c t)"), psa[:],
                            AF.Copy, scale=fscale)
                    else:
                        for c in range(2):
                            nc.scalar.activation(
                                g_sb[:, c], psa[:, TP * c:TP * c + TP],
                                AF.Identity, bias=b2_sb[:, c:c + 1],
                                scale=fscale)
                else:
                    psb = psum2()
                    g_sb = state[i]["g_sb"]
                    for j in range(0, FC, 2):
                        nc.tensor.matmul(
                            psb[:, 0:TP], W28[:, j:j + 2, 2 * P:3 * P],
                            FF_sb[:, j:j + 2], start=(j == 0),
                            stop=(j == FC - 2), perf_mode=PM.DoubleRow)
                    if no_affine:
                        nc.scalar.activation(g_sb[:, 2], psb[:, 0:TP],
                                             AF.Copy, scale=fscale)
                    else:
                        nc.scalar.activation(
                            g_sb[:, 2], psb[:, 0:TP], AF.Identity,
                            bias=b2_sb[:, 2:3], scale=fscale)

            def st_out(i, sp):
                g_sb = state[i]["g_sb"]
                out1_tok = state[i]["out1"]
                if "y_tok" not in state[i]:
                    y_new = p2.tile([P, 4, C], f32, tag="y_tok")
                    state[i]["y_tok"] = y_new
                y_tok = state[i]["y_tok"]
                tp = psum_t()
                for si in range(2):
                    so = 2 * sp + si
                    for mo in range(CC):
                        nc.tensor.matmul(
                            tp[:, TP * si + P * mo:TP * si + P * mo + P],
                            g_sb[:, mo, P * so:P * so + P],
                            ident_bf[:], is_transpose=True)
                nc.vector.tensor_tensor(
                    y_tok[:, 2 * sp:2 * sp + 2],
                    tp[:].rearrange("p (si x) -> p si x", si=2)[:, :, 0:C],
                    out1_tok[:, 2 * sp:2 * sp + 2], OP.add)
                if sp == 1:
                    y_view = y_d[2 * i:2 * i + 2].rearrange(
                        "b (o p) c -> p (b o) c", p=P)
                    nc.sync.dma_start(y_view, y_tok[:])
                    state.pop(i)

            import contextlib
            rep_ctx = (tc.For_i(0, repeat, 1) if repeat > 1
                       else contextlib.nullcontext())
            with rep_ctx:
              # skew-3 pipeline: front pair a = it, attention pair b = it-1,
              # proj/LN2 pair c = it-2, FFN/out pair d = it-3. Dense work is
              # woven between pair b's latency-bound attention steps, and the
              # long proj/LN2 chain runs on 2-back pairs whose inputs are
              # long ready.
              for it in range(n_pairs + 3):
                  a, b, c, d = it, it - 1, it - 2, it - 3
                  ina = a < n_pairs
                  inb = 0 <= b < n_pairs
                  inc = 0 <= c < n_pairs
                  ind = 0 <= d
                  if ina:
                      st_ln1(a)
                  if inb:
                      st_att_sc(b, 0)
                  if ind:
                      st_ffn1(d, (0, 1))
                  if ina:
                      st_qtkt(a)
                  if inb:
                      st_att_sc(b, 1)
                      st_att_av(b, 0)
                  if ind:
                      st_ffn1(d, (2, 3))
                  if inb:
                      st_att_sc(b, 2)
                      st_att_av(b, 1)
                  if ina:
                      st_v(a)
                  if inb:
                      st_att_av(b, 2)
                  if ind:
                      st_ffn1(d, (4, 5))
                  if inc:
                      st_projln2(c)
                  if ind:
                      st_ffn2(d, 0)
                      st_ffn2(d, 1)
                      st_out(d, 0)
                      st_out(d, 1)

    nc.compile()
    return nc


_NC_CACHE = {}


def prep_inputs(inputs):
    """Host-side prep: fp8(x32) weights in [P, ktiles, width] layout plus
    sel2/tri01/ones constants. Returns the non-x input map."""
    import ml_dtypes
    e4m3 = ml_dtypes.float8_e4m3

    def f(k):
        return np.ascontiguousarray(np.asarray(inputs[k], dtype=np.float32))

    def q8(a):
        return np.ascontiguousarray((a * WS).astype(e4m3))

    def chunked(w, width):  # [C_in, width] -> [P, 4, width] padded fp8
        arr = np.zeros((P, 4, width), np.float32)
        arr[:, 0:CC] = w.reshape(CC, P, width).transpose(1, 0, 2)
        return q8(arr)

    Wq, Wk, Wv = f("Wq"), f("Wk"), f("Wv")
    qkv = {}
    for nm, W in (("Wq8", Wq), ("Wk8", Wk), ("Wv8", Wv)):
        arr = np.zeros((P, 4, C), np.float32)
        for h in range(H):
            arr[:, 0:CC, 64 * h:64 * h + 64] = (
                W[h].reshape(CC, P, D).transpose(1, 0, 2))
        qkv[nm] = q8(arr)

    sel2 = np.zeros((P, P), np.float32)
    sel2[64, 0:64] = AVS
    sel2[96, 64:128] = AVS
    tri01 = np.triu(np.ones((P, P), np.float32))
    triA = np.tril(np.ones((P, P), np.float32)).T.astype(ml_dtypes.bfloat16)
    # triA[k, s] = 1 iff k <= s
    triB = np.zeros((P, P), np.float32)
    for t in range(P - 1):
        triB[t + 1, t] = -1e38
    triB = triB.astype(ml_dtypes.bfloat16)

    m = {
        "ln1_g": f("ln1_g"), "ln1_b": f("ln1_b"),
        "ln2_g": f("ln2_g"), "ln2_b": f("ln2_b"),
        # slow-path biases ride pre-scaled to match the device scale system
        "bq": f("bq") * WS, "bk": f("bk") * WS, "bp": f("bp"),
        "b1": f("b1") * WS, "b2": f("b2"),
        "Wp8": chunked(f("Wp"), C),
        "W18": chunked(f("W1"), FF),
        "W28": np.ascontiguousarray(
            (f("W2").reshape(FC, P, C).transpose(1, 0, 2) * WS).astype(e4m3)),
        "bvrow8": q8(f("bv").reshape(1, -1)),
        "ones8": np.ones((1, P), e4m3),
        "sel2": sel2, "tri01": tri01, "triA": triA, "triB": triB,
    }
    m.update(qkv)
    return m


def affine_flags(inputs):
    def z(k):
        return not np.any(np.asarray(inputs[k]))

    no_affine = (z("bq") and z("bk") and z("bp") and z("b1") and z("b2")
                 and z("ln1_b") and z("ln2_b")
                 and np.all(np.asarray(inputs["ln1_g"]) == 1.0)
                 and np.all(np.asarray(inputs["ln2_g"]) == 1.0))
    v_bias = bool(np.any(np.asarray(inputs["bv"])))
    return {"no_affine": no_affine, "v_bias": v_bias}


def kernel(_run_kwargs=None, **inputs) -> np.ndarray:
    run_kwargs = _run_kwargs or {}
    x = np.ascontiguousarray(np.asarray(inputs["x"], dtype=np.float32))
    weights = prep_inputs(inputs)

    flags = affine_flags(inputs)
    key = ("nc", flags["no_affine"], flags["v_bias"])
    if key not in _NC_CACHE:
        _NC_CACHE[key] = build_nc(**flags)
    nc = _NC_CACHE[key]

    in_maps = []
    for c in range(N_CORES):
        m = {"x": x[c * B_LOCAL:(c + 1) * B_LOCAL]}
        m.update(weights)
        in_maps.append(m)

    res = run_bass_kernel_spmd(nc, in_maps, core_ids=list(range(N_CORES)), **run_kwargs)
    y = np.concatenate([r["y"] for r in res.results], axis=0)
    kernel.last_result = res
    return y


# revision 39
# speedup vs baseline: 1.1214x; 1.0418x over previous
"""Trainium2 Bass kernel for nn_Block_82111184765408 (pre-LN transformer block).

B=128, T=256, C=384, H=6, D=64, FF=1536. Data-parallel over batch across 8
NeuronCores (16 batches/core), batches processed in fused pairs (free dim 512).

v6: fp8(e4m3) DoubleRow matmuls for the C/FF contractions (weights quantized
x32 on host), bf16-identity PE transposes, bn_stats layernorm with a quake
bit-trick rsqrt on DVE (keeps the ACT engine on one function set - no
activation-table reloads), causal masking via tiny bf16 PE matmuls that add
-1e38 into the score psum, a compact [P,768] E layout so exp is a single ACT
op per head (attn@V is causal-skipped fp8 singles), parity-column softmax
denominators (rows 64/96) with a per-mo reciprocal + sel-matmul broadcast,
2-bank [P,1024] psum tiles halving ACT/DVE copy op count, the reciprocal
broadcast on a dedicated 1-bank psum pool (relieving main psum rotation),
and a skew-2 software pipeline: pair i's dense QKV work is woven between
pair i-1's latency-bound attention steps with pair i-2's FFN as filler.
"""

import numpy as np

import concourse.bass as bass
import concourse.mybir as mybir
import concourse.tile as tile
from concourse import bacc
from concourse.bass_utils import run_bass_kernel_spmd
from concourse.masks import make_identity

P = 128
B, T, C, H, D = 128, 256, 384, 6, 64
FF = 4 * C
N_CORES = 8
B_LOCAL = B // N_CORES          # 16 batches per core
N_PAIRS = B_LOCAL // 2          # 8 pairs, free dim 512 per pair
TP = 2 * T                      # 512
CC = C // P                     # 3 feature chunks
FC = FF // P                    # 12 ffn chunks
EPS = 1e-5
SCALE = C ** -0.5
WS = 32.0                       # fp8 weight scale
AVS = 4.0                       # AVT scale (baked into sel2)

f32 = mybir.dt.float32
f32r = mybir.dt.float32r
bf16 = mybir.dt.bfloat16
f8 = mybir.dt.float8e4
AF = mybir.ActivationFunctionType
OP = mybir.AluOpType
PM = mybir.MatmulPerfMode


def build_nc(n_pairs=N_PAIRS, debug_outputs=False, repeat=1,
             no_affine=True, v_bias=False):
    nc = bacc.Bacc("TRN2", target_bir_lowering=False, debug=False)

    x_d = nc.declare_dram_parameter("x", [2 * n_pairs, T, C], f32, isOutput=False)
    ln1_g_d = nc.declare_dram_parameter("ln1_g", [C], f32, isOutput=False)
    ln1_b_d = nc.declare_dram_parameter("ln1_b", [C], f32, isOutput=False)
    bk_d = nc.declare_dram_parameter("bk", [H, D], f32, isOutput=False)
    bq_d = nc.declare_dram_parameter("bq", [H, D], f32, isOutput=False)
    bp_d = nc.declare_dram_parameter("bp", [C], f32, isOutput=False)
    ln2_g_d = nc.declare_dram_parameter("ln2_g", [C], f32, isOutput=False)
    ln2_b_d = nc.declare_dram_parameter("ln2_b", [C], f32, isOutput=False)
    b1_d = nc.declare_dram_parameter("b1", [FF], f32, isOutput=False)
    b2_d = nc.declare_dram_parameter("b2", [C], f32, isOutput=False)
    # host-prepared fp8 weights (x32, [P, ktiles, width], zero-padded)
    Wq8_d = nc.declare_dram_parameter("Wq8", [P, 4, C], f8, isOutput=False)
    Wk8_d = nc.declare_dram_parameter("Wk8", [P, 4, C], f8, isOutput=False)
    Wv8_d = nc.declare_dram_parameter("Wv8", [P, 4, C], f8, isOutput=False)
    Wp8_d = nc.declare_dram_parameter("Wp8", [P, 4, C], f8, isOutput=False)
    W18_d = nc.declare_dram_parameter("W18", [P, 4, FF], f8, isOutput=False)
    W28_d = nc.declare_dram_parameter("W28", [P, FC, C], f8, isOutput=False)
    bvrow8_d = nc.declare_dram_parameter("bvrow8", [1, C], f8, isOutput=False)
    ones8_d = nc.declare_dram_parameter("ones8", [1, P], f8, isOutput=False)
    sel2_d = nc.declare_dram_parameter("sel2", [P, P], f32r, isOutput=False)
    tri01_d = nc.declare_dram_parameter("tri01", [P, P], f32, isOutput=False)
    triA_d = nc.declare_dram_parameter("triA", [P, P], mybir.dt.bfloat16, isOutput=False)
    triB_d = nc.declare_dram_parameter("triB", [P, P], mybir.dt.bfloat16, isOutput=False)
    y_d = nc.declare_dram_parameter("y", [2 * n_pairs, T, C], f32, isOutput=True)
    dbg = {}
    if debug_outputs:
        for nm, shp in (("h1T0", [P, TP]), ("QT0", [P, TP]), ("KT0", [P, TP]),
                        ("V0", [P, TP]), ("E0", [P, TP]), ("AVT0", [P, TP]),
                        ("proj0", [P, TP]), ("out1", [P, 4 * C]),
                        ("h2T0", [P, TP]), ("FF0", [P, TP])):
            dbg[nm] = nc.declare_dram_parameter(nm, shp, f32, isOutput=True)

    with tile.TileContext(nc) as tc:
        with tc.tile_pool(name="const", bufs=1) as cst, \
             tc.tile_pool(name="p2", bufs=2) as p2, \
             tc.tile_pool(name="p3", bufs=4) as p3, \
             tc.tile_pool(name="ps2", bufs=3, space="PSUM") as ps2p, \
             tc.tile_pool(name="pst", bufs=1, space="PSUM") as pstp, \
             tc.tile_pool(name="ps1", bufs=1, space="PSUM") as ps1p:

            # ---------- constants ----------
            ident = cst.tile([P, P], f32, tag="ident")
            make_identity(nc, ident[:])
            ident_bf = cst.tile([P, P], bf16, tag="ident_bf")
            nc.vector.tensor_copy(ident_bf[:], ident[:])

            def load_w8(name, dram, kt, width):
                w8 = cst.tile([P, kt, width], f8, tag=f"{name}8")
                nc.sync.dma_start(w8[:], dram[:])
                return w8

            Wq8 = load_w8("Wq", Wq8_d, 4, C)
            Wk8 = load_w8("Wk", Wk8_d, 4, C)
            Wv8 = load_w8("Wv", Wv8_d, 4, C)
            Wp8 = load_w8("Wp", Wp8_d, 4, C)
            W18 = load_w8("W1", W18_d, 4, FF)
            W28 = load_w8("W2", W28_d, FC, C)

            g1_sb = cst.tile([P, CC], f32, tag="g1")
            nc.sync.dma_start(g1_sb[:], ln1_g_d.rearrange("(o p) -> p o", p=P))
            lb1_sb = cst.tile([P, CC], f32, tag="lb1")
            nc.sync.dma_start(lb1_sb[:], ln1_b_d.rearrange("(o p) -> p o", p=P))
            g2_sb = cst.tile([P, CC], f32, tag="g2")
            nc.sync.dma_start(g2_sb[:], ln2_g_d.rearrange("(o p) -> p o", p=P))
            lb2_sb = cst.tile([P, CC], f32, tag="lb2")
            nc.sync.dma_start(lb2_sb[:], ln2_b_d.rearrange("(o p) -> p o", p=P))

            bq_sb = cst.tile([P, CC], f32, tag="bq")
            nc.sync.dma_start(
                bq_sb[:], bq_d.rearrange("h d -> (h d)").rearrange("(o p) -> p o", p=P))
            bk_sb = cst.tile([P, CC], f32, tag="bk")
            nc.sync.dma_start(
                bk_sb[:], bk_d.rearrange("h d -> (h d)").rearrange("(o p) -> p o", p=P))
            bp_sb = cst.tile([P, CC], f32, tag="bp")
            nc.sync.dma_start(bp_sb[:], bp_d.rearrange("(o p) -> p o", p=P))
            b1f_sb = cst.tile([P, FC], f32, tag="b1f")
            nc.sync.dma_start(b1f_sb[:], b1_d.rearrange("(o p) -> p o", p=P))
            b2_sb = cst.tile([P, CC], f32, tag="b2")
            nc.sync.dma_start(b2_sb[:], b2_d.rearrange("(o p) -> p o", p=P))

            bvrow8 = cst.tile([1, C], f8, tag="bvrow8")
            nc.sync.dma_start(bvrow8[:], bvrow8_d[:])
            ones8 = cst.tile([1, P], f8, tag="ones8")
            nc.sync.dma_start(ones8[:], ones8_d[:])

            sel2 = cst.tile([P, P], f32r, tag="sel2")
            nc.sync.dma_start(sel2[:], sel2_d[:])
            tri01 = cst.tile([P, P], f32, tag="tri01")
            nc.sync.dma_start(tri01[:], tri01_d[:])
            triA = cst.tile([P, P], bf16, tag="triA")
            nc.sync.dma_start(triA[:], triA_d[:])
            triB = cst.tile([P, P], bf16, tag="triB")
            nc.sync.dma_start(triB[:], triB_d[:])

            # ---------- per-pair pools (pre-padded slots) ----------
            V_slots = []
            for _ in range(2):
                V_sb = p2.tile([P, 4, H, P], f8, tag="V")
                nc.gpsimd.memset(V_sb[:, :, :, 64:128], 0.0)
                for h in range(H):
                    col = 64 if h % 2 == 0 else 96
                    nc.gpsimd.memset(V_sb[:, :, h, col:col + 1], 1.0)
                V_slots.append(V_sb)

            # compact E: cols [0:256]=sc0-bb0, [256:512]=sc0-bb1,
            # [512:640]=sc1-bb0 (t 128:256), [640:768]=sc1-bb1
            E_slots = []
            for _ in range(4):
                E = p3.tile([P, 768], f8, tag="E")
                E_slots.append(E)
            e_ctr = [0]

            def next_E():
                E = E_slots[e_ctr[0] % 4]
                e_ctr[0] += 1
                return E

            rec_slots = []
            for _ in range(2):
                rc = p2.tile([P, TP], f32r, tag="rec")
                nc.vector.tensor_scalar(
                    rc[:].rearrange("p (a b) -> p a b", a=4),
                    tri01[:, None, :].to_broadcast((P, 4, P)),
                    0.0, 0.0, OP.mult, OP.add)
                rec_slots.append(rc)
            rec_ctr = [0]

            def next_rec():
                rc = rec_slots[rec_ctr[0] % 2]
                rec_ctr[0] += 1
                return rc

            # h1T/h2T/AVT fp8 [P, 4, TP]: 4th k-tile permanently zero.
            hT_slots = {}
            for tag in ("h1T", "h2T", "AVT"):
                hT_slots[tag] = []
                for _ in range(2):
                    t8 = p2.tile([P, 4, TP], f8, tag=tag)
                    nc.gpsimd.memset(t8[:, 3, :], 0.0)
                    hT_slots[tag].append(t8)

            def psum2():  # [P, 1024] f32 - two banks
                return ps2p.tile([P, 2 * TP], f32, tag="mm2", name="mm2")

            def psum_t():  # [P, 1024] bf16 - one bank
                return pstp.tile([P, 2 * TP], bf16, tag="tp", name="tp")

            # ---------- helpers ----------
            def layernorm(src_tok, g_sb, lb_sb, dstT, tagp, gb_engine):
                """src_tok [P,4,C] f32 -> dstT fp8 [P,4,TP] (k-tiles 0:3).
                Stats on DVE (bn_stats), rsqrt+normalize on Pool, transposes
                on PE (bf16), gain/bias fold on gb_engine."""
                st = p2.tile([P, 4, 6], f32, tag=f"{tagp}_st")
                mv = p2.tile([P, 4, 2], f32, tag=f"{tagp}_mv")
                for so in range(4):
                    nc.vector.bn_stats(st[:, so], src_tok[:, so])
                    nc.vector.bn_aggr(mv[:, so], st[:, so])
                # rs ~= rsqrt(var): quake bit-trick plus one Newton step,
                # few tiny DVE ops; keeps Sqrt off the ACT engine so its
                # function table never reloads. eps dropped (var ~ 1 here).
                y0 = p2.tile([P, 4], f32, tag=f"{tagp}_y0")
                i32 = mybir.dt.int32
                nc.vector.tensor_scalar(
                    y0[:].bitcast(i32), mv[:, :, 1].bitcast(i32), 1, None,
                    OP.logical_shift_right)
                nc.vector.tensor_scalar(
                    y0[:].bitcast(i32), y0[:].bitcast(i32), -1, 0x5f3759df,
                    OP.mult, OP.add)
                t1 = p2.tile([P, 4], f32, tag=f"{tagp}_t1")
                nc.vector.tensor_tensor(t1[:], y0[:], y0[:], OP.mult)
                nc.vector.tensor_tensor(t1[:], t1[:], mv[:, :, 1], OP.mult)
                nc.vector.tensor_scalar(t1[:], t1[:], -0.5, 1.5,
                                        OP.mult, OP.add)
                rs = p2.tile([P, 4], f32, tag=f"{tagp}_rs")
                nc.vector.tensor_tensor(rs[:], y0[:], t1[:], OP.mult)
                murs = p2.tile([P, 4], f32, tag=f"{tagp}_murs")
                nc.vector.tensor_tensor(murs[:], mv[:, :, 0], rs[:], OP.mult)
                htok = p2.tile([P, 4, C], bf16, tag=f"{tagp}_htok")
                for so in range(4):
                    nc.vector.tensor_scalar(
                        htok[:, so], src_tok[:, so], rs[:, so:so + 1],
                        murs[:, so:so + 1], OP.mult, OP.subtract)
                # transposes: chunks c0,c1 share a [P,1024] bf16 psum; c2 alone
                tpa = psum_t()
                for c in range(2):
                    for so in range(4):
                        nc.tensor.matmul(
                            tpa[:, TP * c + P * so:TP * c + P * so + P],
                            htok[:, so, P * c:P * c + P],
                            ident_bf[:], is_transpose=True)
                tpb = psum_t()
                for so in range(4):
                    nc.tensor.matmul(
                        tpb[:, P * so:P * so + P],
                        htok[:, so, 2 * P:3 * P],
                        ident_bf[:], is_transpose=True)
                if no_affine:
                    # gains are 1 and biases 0: pure merged copies
                    if gb_engine == "dve":
                        nc.vector.tensor_copy(
                            dstT[:, 0:2].rearrange("p c t -> p (c t)"), tpa[:])
                        nc.vector.tensor_copy(dstT[:, 2], tpb[:, 0:TP])
                    else:
                        nc.scalar.activation(
                            dstT[:, 0:2].rearrange("p c t -> p (c t)"), tpa[:],
                            AF.Copy)
                        nc.scalar.activation(dstT[:, 2], tpb[:, 0:TP], AF.Copy)
                else:
                    for c in range(CC):
                        src_ap = (tpa[:, TP * c:TP * c + TP] if c < 2
                                  else tpb[:, 0:TP])
                        if gb_engine == "dve":
                            nc.vector.tensor_scalar(
                                dstT[:, c], src_ap, g_sb[:, c:c + 1],
                                lb_sb[:, c:c + 1], OP.mult, OP.add)
                        else:
                            nc.scalar.activation(
                                dstT[:, c], src_ap, AF.Identity,
                                bias=lb_sb[:, c:c + 1], scale=g_sb[:, c:c + 1])

            def mm_c4(ps_ap, W8t, xT, col):
                """ps_ap [P,512] += W8t[:, :, col*128:+128].T @ xT, 4 k-tiles
                via 2 DoubleRows."""
                for j in (0, 2):
                    nc.tensor.matmul(
                        ps_ap, W8t[:, j:j + 2, P * col:P * col + P],
                        xT[:, j:j + 2], start=(j == 0), stop=(j == 2),
                        perf_mode=PM.DoubleRow)

            # ---------- software-pipelined pair loop ----------
            # Emission order interleaves pair i's latency-bound stages (LN,
            # attention chain) with pair i-1's dense FFN work so every
            # engine's in-order stream stays fed.
            state = {}

            def st_ln1(i):
                x_view = x_d[2 * i:2 * i + 2].rearrange(
                    "b (o p) c -> p (b o) c", p=P)
                x_tok = p2.tile([P, 4, C], f32, tag="x_tok", bufs=3)
                nc.sync.dma_start(x_tok[:], x_view)
                h1T = hT_slots["h1T"][i % 2]
                layernorm(x_tok, g1_sb, lb1_sb, h1T, "ln1", "dve")
                state[i] = {"x_tok": x_tok, "h1T": h1T}

            def st_qtkt(i):
                h1T = state[i]["h1T"]
                QT = p2.tile([P, CC, TP], f8, tag="QT")
                KT = p2.tile([P, CC, TP], f8, tag="KT")
                for (W8t, b_sb, dst) in ((Wq8, bq_sb, QT), (Wk8, bk_sb, KT)):
                    psa = psum2()
                    mm_c4(psa[:, 0:TP], W8t, h1T, 0)
                    mm_c4(psa[:, TP:2 * TP], W8t, h1T, 1)
                    psb = psum2()
                    mm_c4(psb[:, 0:TP], W8t, h1T, 2)
                    # QT/KT hold WS*Q / WS*K; exp absorbs 1/WS^2
                    if no_affine:
                        if dst is QT:
                            nc.scalar.activation(
                                dst[:, 0:2].rearrange("p c t -> p (c t)"),
                                psa[:], AF.Copy)
                            nc.scalar.activation(dst[:, 2], psb[:, 0:TP],
                                                 AF.Copy)
                        else:
                            nc.vector.tensor_copy(
                                dst[:, 0:2].rearrange("p c t -> p (c t)"),
                                psa[:])
                            nc.vector.tensor_copy(dst[:, 2], psb[:, 0:TP])
                    else:
                        for c in range(CC):
                            src_ap = (psa[:, TP * c:TP * c + TP] if c < 2
                                      else psb[:, 0:TP])
                            if dst is QT:
                                nc.scalar.activation(
                                    dst[:, c], src_ap, AF.Identity,
                                    bias=b_sb[:, c:c + 1])
                            else:
                                nc.vector.tensor_scalar(
                                    dst[:, c], src_ap, 1.0,
                                    b_sb[:, c:c + 1], OP.mult, OP.add)
                state[i]["QT"] = QT
                state[i]["KT"] = KT

            def st_v(i):
                h1T = state[i]["h1T"]
                V_sb = V_slots[i % 2]
                for tg in range(2):
                    ps = psum2()
                    for ti in range(2):
                        to = 2 * tg + ti
                        base = TP * ti
                        for j in (0, 2):
                            nc.tensor.matmul(
                                ps[:, base:base + C],
                                h1T[:, j:j + 2, P * to:P * to + P],
                                Wv8[:, j:j + 2], start=(j == 0),
                                stop=(j == 2 and not v_bias),
                                perf_mode=PM.DoubleRow)
                        if v_bias:
                            nc.tensor.matmul(ps[:, base:base + C], ones8[:],
                                             bvrow8[:], start=False, stop=True)
                    nc.scalar.activation(
                        V_sb[:, 2 * tg:2 * tg + 2, :, 0:64],
                        ps[:].rearrange("p (ti x) -> p ti x", ti=2)[:, :, 0:C]
                            .rearrange("p ti (h d) -> p ti h d", h=H),
                        AF.Copy)
                state[i]["V"] = V_sb

            def st_att_sc(i, mo):
                QT, KT = state[i]["QT"], state[i]["KT"]
                Es = {}
                for half in range(2):
                    rows = slice(64 * half, 64 * half + 64)
                    # scoresT in one [P,1024]: sc0 cols 0:512, sc1 512:768
                    sps = psum2()
                    for bb in range(2):
                        nc.tensor.matmul(
                            sps[:, 256 * bb:256 * bb + 256],
                            QT[rows, mo, 256 * bb:256 * bb + 128],
                            KT[rows, mo, 256 * bb:256 * bb + 256],
                            start=True, stop=True)
                        nc.tensor.matmul(
                            sps[:, TP + 128 * bb:TP + 128 * bb + 128],
                            QT[rows, mo, 256 * bb + 128:256 * bb + 256],
                            KT[rows, mo, 256 * bb + 128:256 * bb + 256],
                            start=True, stop=True)
                    # additive causal mask on the four diagonal blocks:
                    # psum += triA.T @ triB = -1e38 where s > t
                    for c0 in (0, 256, TP, TP + 128):
                        nc.tensor.matmul(
                            sps[:, c0:c0 + 128], triA[:], triB[:],
                            start=False, stop=True, skip_group_check=True)
                    E = next_E()
                    nc.scalar.activation(E[:], sps[:, 0:768], AF.Exp,
                                         scale=SCALE / (WS * WS))
                    Es[half] = E
                state[i][("Es", mo)] = Es

            def st_att_av(i, mo):
                V_sb = state[i]["V"]
                AVT = hT_slots["AVT"][i % 2]
                state[i]["AVT"] = AVT
                Es = state[i].pop(("Es", mo))
                rec = next_rec()
                av2 = psum2()
                for half in range(2):
                    h = 2 * mo + half
                    for bb in range(2):
                        base = TP * half + 256 * bb
                        nc.tensor.matmul(
                            av2[0:97, base:base + 256],
                            V_sb[:, 2 * bb, h, 0:97],
                            Es[half][:, 256 * bb:256 * bb + 256],
                            start=True, stop=False, skip_group_check=True)
                        nc.tensor.matmul(
                            av2[0:97, base + 128:base + 256],
                            V_sb[:, 2 * bb + 1, h, 0:97],
                            Es[half][:, 512 + 128 * bb:512 + 128 * bb + 128],
                            start=False, stop=True, skip_group_check=True)
                with nc.allow_low_precision(reason="softmax recip"):
                    nc.vector.reciprocal(rec[64:65, :], av2[64:65, 0:TP])
                    nc.vector.reciprocal(rec[96:97, :], av2[96:97, TP:2 * TP])
                rps2 = ps1p.tile([P, TP], f32, tag="rps", name="rps")
                nc.tensor.matmul(rps2[:], sel2[64:97, :],
                                 rec[64:97, :], start=True, stop=True)
                rps_sb = p2.tile([P, TP], bf16, tag="rps_sb")
                nc.scalar.activation(rps_sb[:], rps2[:], AF.Copy)
                for half in range(2):
                    rows = slice(64 * half, 64 * half + 64)
                    nc.vector.tensor_tensor(
                        AVT[rows, mo], av2[0:64, TP * half:TP * half + TP],
                        rps_sb[rows, :], OP.mult)

            def st_projln2(i):
                AVT = state[i]["AVT"]
                x_tok = state[i]["x_tok"]
                proj_sb = p2.tile([P, CC, TP], bf16, tag="proj_sb")
                psa = psum2()
                mm_c4(psa[:, 0:TP], Wp8, AVT, 0)
                mm_c4(psa[:, TP:2 * TP], Wp8, AVT, 1)
                psb = psum2()
                mm_c4(psb[:, 0:TP], Wp8, AVT, 2)
                pscale = 1.0 / (WS * WS * AVS)
                if no_affine:
                    nc.scalar.activation(
                        proj_sb[:, 0:2].rearrange("p c t -> p (c t)"), psa[:],
                        AF.Copy, scale=pscale)
                    nc.scalar.activation(
                        proj_sb[:, 2], psb[:, 0:TP], AF.Copy, scale=pscale)
                else:
                    for c in range(CC):
                        src_ap = (psa[:, TP * c:TP * c + TP] if c < 2
                                  else psb[:, 0:TP])
                        nc.scalar.activation(
                            proj_sb[:, c], src_ap, AF.Identity,
                            bias=bp_sb[:, c:c + 1], scale=pscale)
                out1_tok = p2.tile([P, 4, C], f32, tag="out1_tok")
                for sp in range(2):
                    tp = psum_t()
                    for si in range(2):
                        so = 2 * sp + si
                        for mo in range(CC):
                            nc.tensor.matmul(
                                tp[:, TP * si + P * mo:TP * si + P * mo + P],
                                proj_sb[:, mo, P * so:P * so + P],
                                ident_bf[:], is_transpose=True)
                    nc.vector.tensor_tensor(
                        out1_tok[:, 2 * sp:2 * sp + 2],
                        tp[:].rearrange("p (si x) -> p si x", si=2)[:, :, 0:C],
                        x_tok[:, 2 * sp:2 * sp + 2], OP.add)
                state[i]["out1"] = out1_tok
                h2T = hT_slots["h2T"][i % 2]
                layernorm(out1_tok, g2_sb, lb2_sb, h2T, "ln2", "act")
                state[i]["h2T"] = h2T

            def st_ffn1(i, fps):
                h2T = state[i]["h2T"]
                if "FF" not in state[i]:
                    FF_new = p2.tile([P, FC, TP], f8, tag="FF_sb")
                    state[i]["FF"] = FF_new
                FF_sb = state[i]["FF"]
                for fp in fps:
                    ps = psum2()
                    mm_c4(ps[:, 0:TP], W18, h2T, 2 * fp)
                    mm_c4(ps[:, TP:2 * TP], W18, h2T, 2 * fp + 1)
                    if no_affine:
                        nc.scalar.activation(
                            FF_sb[:, 2 * fp:2 * fp + 2].rearrange(
                                "p c t -> p (c t)"),
                            ps[:], AF.Relu)
                    else:
                        for ci in range(2):
                            fo = 2 * fp + ci
                            nc.scalar.activation(
                                FF_sb[:, fo], ps[:, TP * ci:TP * ci + TP],
                                AF.Relu, bias=b1f_sb[:, fo:fo + 1])

            def st_ffn2(i, part):
                FF_sb = state[i]["FF"]
                fscale = 1.0 / (WS * WS)
                if part == 0:
                    psa = psum2()
                    g_new = p2.tile([P, CC, TP], bf16, tag="g_sb")
                    state[i]["g_sb"] = g_new
                    for mo in range(2):
                        for j in range(0, FC, 2):
                            nc.tensor.matmul(
                                psa[:, TP * mo:TP * mo + TP],
                                W28[:, j:j + 2, P * mo:P * mo + P],
                                FF_sb[:, j:j + 2], start=(j == 0),
                                stop=(j == FC - 2), perf_mode=PM.DoubleRow)
                    g_sb = state[i]["g_sb"]
                    if no_affine:
                        nc.scalar.activation(
                            g_sb[:, 0:2].rearrange("p c t -> p (c t)"), psa[:],
                            AF.Copy, scale=fscale)
                    else:
                        for c in range(2):
                            nc.scalar.activation(
                                g_sb[:, c], psa[:, TP * c:TP * c + TP],
                                AF.Identity, bias=b2_sb[:, c:c + 1],
                                scale=fscale)
                else:
                    psb = psum2()
                    g_sb = state[i]["g_sb"]
                    for j in range(0, FC, 2):
                        nc.tensor.matmul(
                            psb[:, 0:TP], W28[:, j:j + 2, 2 * P:3 * P],
                            FF_sb[:, j:j + 2], start=(j == 0),
                            stop=(j == FC - 2), perf_mode=PM.DoubleRow)
                    if no_affine:
                        nc.scalar.activation(g_sb[:, 2], psb[:, 0:TP],
                                             AF.Copy, scale=fscale)
                    else:
                        nc.scalar.activation(
                            g_sb[:, 2], psb[:, 0:TP], AF.Identity,
                            bias=b2_sb[:, 2:3], scale=fscale)

            def st_out(i, sp):
                g_sb = state[i]["g_sb"]
                out1_tok = state[i]["out1"]
                if "y_tok" not in state[i]:
                    y_new = p2.tile([P, 4, C], f32, tag="y_tok")
                    state[i]["y_tok"] = y_new
                y_tok = state[i]["y_tok"]
                tp = psum_t()
                for si in range(2):
                    so = 2 * sp + si
                    for mo in range(CC):
                        nc.tensor.matmul(
                            tp[:, TP * si + P * mo:TP * si + P * mo + P],
                            g_sb[:, mo, P * so:P * so + P],
                            ident_bf[:], is_transpose=True)
                nc.vector.tensor_tensor(
                    y_tok[:, 2 * sp:2 * sp + 2],
                    tp[:].rearrange("p (si x) -> p si x", si=2)[:, :, 0:C],
                    out1_tok[:, 2 * sp:2 * sp + 2], OP.add)
                if sp == 1:
                    y_view = y_d[2 * i:2 * i + 2].rearrange(
                        "b (o p) c -> p (b o) c", p=P)
                    nc.sync.dma_start(y_view, y_tok[:])
                    state.pop(i)

            import contextlib
            rep_ctx = (tc.For_i(0, repeat, 1) if repeat > 1
                       else contextlib.nullcontext())
            with rep_ctx:
              # skew-3 pipeline: front pair a = it, attention pair b = it-1,
              # proj/LN2 pair c = it-2, FFN/out pair d = it-3. Dense work is
              # woven between pair b's latency-bound attention steps, and the
              # long proj/LN2 chain runs on 2-back pairs whose inputs are
              # long ready.
              for it in range(n_pairs + 3):
                  a, b, c, d = it, it - 1, it - 2, it - 3
                  ina = a < n_pairs
                  inb = 0 <= b < n_pairs
                  inc = 0 <= c < n_pairs
                  ind = 0 <= d
                  if ina:
                      st_ln1(a)
                  if inb:
                      st_att_sc(b, 0)
                  if ind:
                      st_ffn1(d, (0, 1))
                  if ina:
                      st_qtkt(a)
                  if inb:
                      st_att_sc(b, 1)
                      st_att_av(b, 0)
                  if ind:
                      st_ffn1(d, (2, 3))
                  if inc:
                      st_projln2(c)
                  if inb:
                      st_att_sc(b, 2)
                      st_att_av(b, 1)
                  if ina:
                      st_v(a)
                  if inb:
                      st_att_av(b, 2)
                  if ind:
                      st_ffn1(d, (4, 5))
                  if ind:
                      st_ffn2(d, 0)
                      st_ffn2(d, 1)
                      st_out(d, 0)
                      st_out(d, 1)

    nc.compile()
    return nc


_NC_CACHE = {}


def prep_inputs(inputs):
    """Host-side prep: fp8(x32) weights in [P, ktiles, width] layout plus
    sel2/tri01/ones constants. Returns the non-x input map."""
    import ml_dtypes
    e4m3 = ml_dtypes.float8_e4m3

    def f(k):
        return np.ascontiguousarray(np.asarray(inputs[k], dtype=np.float32))

    def q8(a):
        return np.ascontiguousarray((a * WS).astype(e4m3))

    def chunked(w, width):  # [C_in, width] -> [P, 4, width] padded fp8
        arr = np.zeros((P, 4, width), np.float32)
        arr[:, 0:CC] = w.reshape(CC, P, width).transpose(1, 0, 2)
        return q8(arr)

    Wq, Wk, Wv = f("Wq"), f("Wk"), f("Wv")
    qkv = {}
    for nm, W in (("Wq8", Wq), ("Wk8", Wk), ("Wv8", Wv)):
        arr = np.zeros((P, 4, C), np.float32)
        for h in range(H):
            arr[:, 0:CC, 64 * h:64 * h + 64] = (
                W[h].reshape(CC, P, D).transpose(1, 0, 2))
        qkv[nm] = q8(arr)

    sel2 = np.zeros((P, P), np.float32)
    sel2[64, 0:64] = AVS
    sel2[96, 64:128] = AVS
    tri01 = np.triu(np.ones((P, P), np.float32))
    triA = np.tril(np.ones((P, P), np.float32)).T.astype(ml_dtypes.bfloat16)
    # triA[k, s] = 1 iff k <= s
    triB = np.zeros((P, P), np.float32)
    for t in range(P - 1):
        triB[t + 1, t] = -1e38
    triB = triB.astype(ml_dtypes.bfloat16)

    m = {
        "ln1_g": f("ln1_g"), "ln1_b": f("ln1_b"),
        "ln2_g": f("ln2_g"), "ln2_b": f("ln2_b"),
        # slow-path biases ride pre-scaled to match the device scale system
        "bq": f("bq") * WS, "bk": f("bk") * WS, "bp": f("bp"),
        "b1": f("b1") * WS, "b2": f("b2"),
        "Wp8": chunked(f("Wp"), C),
        "W18": chunked(f("W1"), FF),
        "W28": np.ascontiguousarray(
            (f("W2").reshape(FC, P, C).transpose(1, 0, 2) * WS).astype(e4m3)),
        "bvrow8": q8(f("bv").reshape(1, -1)),
        "ones8": np.ones((1, P), e4m3),
        "sel2": sel2, "tri01": tri01, "triA": triA, "triB": triB,
    }
    m.update(qkv)
    return m


def affine_flags(inputs):
    def z(k):
        return not np.any(np.asarray(inputs[k]))

    no_affine = (z("bq") and z("bk") and z("bp") and z("b1") and z("b2")
                 and z("ln1_b") and z("ln2_b")
                 and np.all(np.asarray(inputs["ln1_g"]) == 1.0)
                 and np.all(np.asarray(inputs["ln2_g"]) == 1.0))
    v_bias = bool(np.any(np.asarray(inputs["bv"])))
    return {"no_affine": no_affine, "v_bias": v_bias}


def kernel(_run_kwargs=None, **inputs) -> np.ndarray:
    run_kwargs = _run_kwargs or {}
    x = np.ascontiguousarray(np.asarray(inputs["x"], dtype=np.float32))
    weights = prep_inputs(inputs)

    flags = affine_flags(inputs)
    key = ("nc", flags["no_affine"], flags["v_bias"])
    if key not in _NC_CACHE:
        _NC_CACHE[key] = build_nc(**flags)
    nc = _NC_CACHE[key]

    in_maps = []
    for c in range(N_CORES):
        m = {"x": x[c * B_LOCAL:(c + 1) * B_LOCAL]}
        m.update(weights)
        in_maps.append(m)

    res = run_bass_kernel_spmd(nc, in_maps, core_ids=list(range(N_CORES)), **run_kwargs)
    y = np.concatenate([r["y"] for r in res.results], axis=0)
    kernel.last_result = res
    return y
